# revision 3
# baseline (speedup 1.0000x reference)
"""Bahdanau additive-attention pooling kernel for Trainium2 (Bass/Tile).

Math (per batch b):
    q = x @ Wt            [L, U]
    k = x @ Wx            [L, U]
    e[i,j] = sum_u Wa[u] * tanh(q[i,u] + k[j,u] + bh[u])   (+ ba, dropped:
                                                            softmax shift-inv)
    a = softmax_j(e)
    v = a @ x             [L, D]

Shapes: B=4, L=1024, D=256, U=32.  8 cores = 4 batches x 2 query-halves
(512 queries/core, all 1024 keys).

Per-core layout trick: partitions = (ii, u) with ii in 0..3 (4 queries per
"group") and u in 0..31 -> 128 partitions.  A [128, 1024] SBUF tile K4 holds
k^T (+0) replicated 4x across the ii blocks.  For query group g the ScalarE
computes
    H = tanh(K4 + Qbias[:, g])      (per-partition bias = q[4g+ii, u]+bh[u])
in ONE activation pass over all 1024 keys.  The PE then contracts over u with
a block-diagonal [128, 4] Wa matrix as the MOVING operand and H key-chunks as
the STATIONARY operand, producing e^T chunks [128 keys, 4 queries] at PSUM
partition base 0 (PE outputs must start at partition 0/32/64/96).  After 32
groups a [128k, 8 chunks, 128q] PSUM tile of transposed logits is complete.
exp on ScalarE -> P^T in SBUF; then P^T chunks are the stationary operand of
    [v | rowsum] = P^T.T @ [x | 1]
so the softmax denominator falls out of the same matmul via a ones-column
appended to x.  Finally v * (1/rowsum) on VectorE -> DMA out.
"""

import os

import numpy as np

import concourse.bass as bass
import concourse.mybir as mybir
import concourse.tile as tile
from concourse import bacc
from concourse.bass import ds, ts

B, L, D, U = 4, 1024, 256, 32
NCORES = 8
HALVES = 2                      # query-dim split per batch
LQ = L // HALVES                # 512 queries per core
QG = 4                          # queries per tanh group (QG*U = 128)
NGRP = LQ // QG                 # 128 groups per core
QB = 128                        # query block = softmax granularity
NQB = LQ // QB                  # 4
NJC = L // 128                  # 8 key chunks
NDC = D // 128                  # 2 contraction chunks
DP1 = D + 1                     # x plus ones column

F32 = mybir.dt.float32
AF = mybir.ActivationFunctionType

MM_DTYPE = mybir.dt.float32     # plain fp32 first; float32r needs rounded producers


def _mm(ap):
    return ap.bitcast(MM_DTYPE) if MM_DTYPE != F32 else ap


def build_kernel(nc: bass.Bass):
    x_d = nc.dram_tensor("x", [L, D], F32, kind="ExternalInput")
    xq_d = nc.dram_tensor("xq", [LQ, D], F32, kind="ExternalInput")
    wt_d = nc.dram_tensor("wt", [D, U], F32, kind="ExternalInput")
    wx_d = nc.dram_tensor("wx", [D, U], F32, kind="ExternalInput")
    bh4_d = nc.dram_tensor("bh4", [128, 1], F32, kind="ExternalInput")
    waexp_d = nc.dram_tensor("waexp", [128, QG], F32, kind="ExternalInput")
    ident_d = nc.dram_tensor("ident", [128, 128], F32, kind="ExternalInput")
    out_d = nc.dram_tensor("out", [LQ, D], F32, kind="ExternalOutput")

    with tile.TileContext(nc) as tc:
        with tc.tile_pool(name="const", bufs=1) as cpool:
            x_sb = cpool.tile([128, NJC, DP1], F32)     # [x | 1]
            xq_sb = cpool.tile([128, NQB, D], F32)
            xT_sb = cpool.tile([128, NDC, L], F32)
            xqT_sb = cpool.tile([128, NDC, LQ], F32)
            wt_sb = cpool.tile([128, NDC, U], F32)
            wx_sb = cpool.tile([128, NDC, U], F32)
            bh4_sb = cpool.tile([128, 1], F32)
            waexp_sb = cpool.tile([128, QG], F32)
            ident_sb = cpool.tile([128, 128], F32)
            k4_sb = cpool.tile([128, L], F32)
            qbias_sb = cpool.tile([128, NGRP], F32)
            recip_sb = cpool.tile([128, NQB], F32)

            nc.sync.dma_start(
                x_sb[:, :, 0:D], x_d.ap().rearrange("(c p) d -> p c d", p=128)
            )
            nc.vector.memset(x_sb[:, :, D], 1.0)
            nc.sync.dma_start(xq_sb[:], xq_d.ap().rearrange("(c p) d -> p c d", p=128))
            nc.sync.dma_start(wt_sb[:], wt_d.ap().rearrange("(c p) u -> p c u", p=128))
            nc.sync.dma_start(wx_sb[:], wx_d.ap().rearrange("(c p) u -> p c u", p=128))
            nc.sync.dma_start(bh4_sb[:], bh4_d.ap())
            nc.sync.dma_start(waexp_sb[:], waexp_d.ap())
            nc.sync.dma_start(ident_sb[:], ident_d.ap())

            # ---- prologue: x^T, xq^T via PE; kT/qT matmuls; K4 + Qbias ----
            with (
                tc.tile_pool(name="ptr", bufs=2, space="PSUM") as ptr,
                tc.tile_pool(name="pkq", bufs=1, space="PSUM") as pkq,
            ):
                for jc in range(NJC):
                    for dc in range(NDC):
                        tr = ptr.tile([128, 128], F32)
                        nc.tensor.transpose(
                            tr[:], x_sb[:, jc, ds(dc * 128, 128)], ident_sb[:]
                        )
                        nc.vector.tensor_copy(xT_sb[:, dc, ts(jc, 128)], tr[:])
                for jc in range(NQB):
                    for dc in range(NDC):
                        tr = ptr.tile([128, 128], F32)
                        nc.tensor.transpose(
                            tr[:], xq_sb[:, jc, ds(dc * 128, 128)], ident_sb[:]
                        )
                        nc.vector.tensor_copy(xqT_sb[:, dc, ts(jc, 128)], tr[:])

                # kT replicated 4x across partition blocks: kt4[(ii,u), j]
                kt4 = pkq.tile([128, L], F32)
                for ii in range(QG):
                    for n in range(L // 512):
                        for dc in range(NDC):
                            nc.tensor.matmul(
                                kt4[ds(ii * U, U), ds(n * 512, 512)],
                                _mm(wx_sb[:, dc, :]),
                                _mm(xT_sb[:, dc, ds(n * 512, 512)]),
                                start=(dc == 0),
                                stop=(dc == NDC - 1),
                                tile_position=(0, ii * U),
                            )
                nc.vector.tensor_copy(k4_sb[:], kt4[:])

                # qT replicated 4x: qt4[(ii,u), i] ; then strided-select the
                # per-group bias column: Qbias[(ii,u), g] = q[4g+ii, u] + bh[u]
                qt4 = pkq.tile([128, LQ], F32)
                for ii in range(QG):
                    for dc in range(NDC):
                        nc.tensor.matmul(
                            qt4[ds(ii * U, U), :],
                            _mm(wt_sb[:, dc, :]),
                            _mm(xqT_sb[:, dc, :]),
                            start=(dc == 0),
                            stop=(dc == NDC - 1),
                            tile_position=(0, ii * U),
                        )
                qt4_r = qt4.rearrange("p (g f) -> p g f", f=QG)
                for ii in range(QG):
                    nc.vector.tensor_scalar_add(
                        qbias_sb[ds(ii * U, U), :],
                        qt4_r[ds(ii * U, U), :, ii],
                        bh4_sb[ds(ii * U, U), :],
                    )

            # ---- main loop ----
            with (
                tc.tile_pool(name="hpool", bufs=3) as hpool,
                tc.tile_pool(name="ppool", bufs=2) as ppool,
                tc.tile_pool(name="vpool", bufs=2) as vpool,
                tc.tile_pool(name="pe", bufs=2, space="PSUM") as pe_e,
                tc.tile_pool(name="pv", bufs=2, space="PSUM") as pe_v,
            ):
                out_r = out_d.ap().rearrange("(c p) d -> c p d", p=128)
                for qb in range(NQB):
                    # eT_ps[p, jc, i] = e[qb*128 + i, jc*128 + p]
                    eT_ps = pe_e.tile([128, NJC, QB], F32)
                    for qg in range(QB // QG):  # 32 groups -> 128 query cols
                        g = qb * (QB // QG) + qg
                        h = hpool.tile([128, L], F32)
                        nc.scalar.activation(
                            h[:], k4_sb[:], AF.Tanh,
                            bias=qbias_sb[:, ds(g, 1)], scale=1.0,
                        )
                        for jc in range(NJC):
                            nc.tensor.matmul(
                                eT_ps[:, jc, ds(qg * QG, QG)],
                                _mm(h[:, ts(jc, 128)]),
                                _mm(waexp_sb[:]),
                                start=True,
                                stop=True,
                            )
                    # softmax numerator (logits bounded by ||Wa||_1 ~ 4.5, so
                    # no max-subtraction needed)
                    pT = ppool.tile([128, NJC, QB], F32)
                    nc.scalar.activation(pT[:], eT_ps[:], AF.Exp)
                    # [v | rowsum] = sum_jc  P^T[jc].T @ [x[jc] | 1]
                    v_ps = pe_v.tile([128, DP1], F32)
                    for jc in range(NJC):
                        nc.tensor.matmul(
                            v_ps[:],
                            _mm(pT[:, jc, :]),
                            _mm(x_sb[:, jc, :]),
                            start=(jc == 0),
                            stop=(jc == NJC - 1),
                        )
                    nc.vector.reciprocal(recip_sb[:, ds(qb, 1)], v_ps[:, ds(D, 1)])
                    v_sb = vpool.tile([128, D], F32)
                    nc.vector.tensor_scalar_mul(
                        v_sb[:], v_ps[:, 0:D], recip_sb[:, ds(qb, 1)]
                    )
                    nc.sync.dma_start(out_r[qb], v_sb[:])

    return nc


_NC_CACHE: dict = {}


def get_compiled_nc():
    if "nc" not in _NC_CACHE:
        nc = bacc.Bacc("TRN2", target_bir_lowering=False, debug=False)
        build_kernel(nc)
        nc.compile()
        _NC_CACHE["nc"] = nc
    return _NC_CACHE["nc"]


def make_in_maps(inputs_np, Wt, Wx, bh, Wa):
    waexp = np.zeros((128, QG), np.float32)
    for ii in range(QG):
        waexp[ii * U : (ii + 1) * U, ii] = Wa[:, 0]
    bh4 = np.tile(bh.reshape(-1), QG).reshape(128, 1).astype(np.float32)
    ident = np.eye(128, dtype=np.float32)
    in_maps = []
    for c in range(NCORES):
        b, half = divmod(c, HALVES)
        in_maps.append(
            {
                "x": np.ascontiguousarray(inputs_np[b]),
                "xq": np.ascontiguousarray(inputs_np[b, half * LQ : (half + 1) * LQ]),
                "wt": Wt,
                "wx": Wx,
                "bh4": bh4,
                "waexp": waexp,
                "ident": ident,
            }
        )
    return in_maps


def kernel(**inputs) -> np.ndarray:
    x = np.asarray(inputs["inputs"], dtype=np.float32)
    Wt = np.ascontiguousarray(np.asarray(inputs["Wt"], np.float32))
    Wx = np.ascontiguousarray(np.asarray(inputs["Wx"], np.float32))
    bh = np.asarray(inputs["bh"], np.float32)
    Wa = np.asarray(inputs["Wa"], np.float32)
    # inputs["ba"] shifts every logit equally; softmax is shift-invariant.

    from concourse.bass_utils import run_bass_kernel_spmd

    nc = get_compiled_nc()
    in_maps = make_in_maps(x, Wt, Wx, bh, Wa)
    res = run_bass_kernel_spmd(nc, in_maps, list(range(NCORES)))
    kernel._last_results = res  # type: ignore[attr-defined]

    out = np.empty((B, L, D), np.float32)
    for c in range(NCORES):
        b, half = divmod(c, HALVES)
        out[b, half * LQ : (half + 1) * LQ] = res.results[c]["out"]
    return out


# revision 4
# speedup vs baseline: 5269.9016x; 5269.9016x over previous
"""Bahdanau additive-attention pooling for TRN2 — v2 layout.

Per core (4 batches x 2 query-halves = 8 cores): 512 queries, 1024 keys.

Partition layout p = 32*uu + ii:  ii in 0..31 = query-within-group,
uu in 0..3 = u-within-slice; u-slices us in 0..7 cover U=32 (u = 4*us+uu).

  K4[us][p, j]  = k[j, 4us+uu]          (built by PE from host-replicated Wx4)
  Qb[us][p, g]  = q[32g+ii, 4us+uu]+bh  (via DRAM round-trip gather of qT)
  S = K4[us] + Qb[us][:,g]              (VectorE tensor_scalar, 2x fp32 mode)
  H = tanh(S)                           (ScalarE, batched 4 slices/instr)
  e[32c:32c+32, :] += wa32[us].T @ H    (PE, M=32 col-tiled at base 32c,
                                         8 accumulating matmuls contract u)
After 4 groups a [128q, 1024k] PSUM logit tile is complete (query block qb).
exp (+row sums via accum_out) -> P; PE-transpose P chunks -> aT; v = aT.T @ x
accumulated over key chunks; scale by 1/rowsum; DMA out.

ba is dropped (softmax shift-invariance); no max-subtraction needed since
|e| <= ||Wa||_1 ~ 4.5.
"""

import numpy as np

import concourse.bass as bass
import concourse.mybir as mybir
import concourse.tile as tile
from concourse import bacc
from concourse.bass import ds, ts

B, L, D, U = 4, 1024, 256, 32
NCORES = 8
HALVES = 2
LQ = L // HALVES                # 512 queries per core
GQ = 32                         # queries per group
NGRP = LQ // GQ                 # 16 groups
NUS = 8                         # u-slices (4 u's each)
USB = 4                         # u-slices per tanh batch
QB = 128                        # query block (softmax granularity)
NQB = LQ // QB                  # 4
NJC = L // 128                  # 8 key chunks
NDC = D // 128                  # 2 contraction chunks

F32 = mybir.dt.float32
F32R = mybir.dt.float32r
F16 = mybir.dt.float16
AF = mybir.ActivationFunctionType


def build_kernel(nc: bass.Bass):
    x_d = nc.dram_tensor("x", [L, D], F32R, kind="ExternalInput")
    xq_d = nc.dram_tensor("xq", [LQ, D], F32R, kind="ExternalInput")
    wt_d = nc.dram_tensor("wt", [D, U], F32R, kind="ExternalInput")
    wx4_d = nc.dram_tensor("wx4", [D, NUS, 128], F32R, kind="ExternalInput")
    wa32_d = nc.dram_tensor("wa32", [NUS, 128, GQ], F16, kind="ExternalInput")
    bh_d = nc.dram_tensor("bh", [U, 1], F32, kind="ExternalInput")
    ident_d = nc.dram_tensor("ident", [128, 128], F32R, kind="ExternalInput")
    out_d = nc.dram_tensor("out", [LQ, D], F32, kind="ExternalOutput")
    qtb_d = nc.dram_tensor("qtb", [U, LQ], F32)  # scratch for the Qb gather

    with tile.TileContext(nc) as tc:
        with tc.tile_pool(name="const", bufs=1) as cpool:
            x_sb = cpool.tile([128, NJC, D], F32R)
            xq_sb = cpool.tile([128, NQB, D], F32R)
            xT_sb = cpool.tile([128, NDC, L], F32R)
            xqT_sb = cpool.tile([128, NDC, LQ], F32R)
            wt_sb = cpool.tile([128, NDC, U], F32R)
            wx4_sb = cpool.tile([128, NDC, NUS, 128], F32R)
            wa32_sb = cpool.tile([128, NUS, GQ], F16)
            bh_sb = cpool.tile([U, 1], F32)
            ident_sb = cpool.tile([128, 128], F32R)
            k4_sb = cpool.tile([128, NUS, L], F32)
            qtb_sb = cpool.tile([U, LQ], F32)
            qb_sb = cpool.tile([128, NUS, NGRP], F32)
            sums_sb = cpool.tile([128, NQB], F32)
            recip_sb = cpool.tile([128, NQB], F32)

            # small/critical DMAs first; 1MB wx4 split per-us and last
            nc.sync.dma_start(ident_sb[:], ident_d.ap())
            nc.sync.dma_start(bh_sb[:], bh_d.ap())
            nc.sync.dma_start(wt_sb[:], wt_d.ap().rearrange("(c p) u -> p c u", p=128))
            nc.sync.dma_start(wa32_sb[:], wa32_d.ap().rearrange("us p m -> p us m"))
            xq_r = xq_d.ap().rearrange("(c p) d -> c p d", p=128)
            for jc in range(NQB):
                nc.sync.dma_start(xq_sb[:, jc, :], xq_r[jc])
            x_r = x_d.ap().rearrange("(c p) d -> c p d", p=128)
            for jc in range(NJC):
                nc.sync.dma_start(x_sb[:, jc, :], x_r[jc])
            wx4_r = wx4_d.ap().rearrange("(c p) us m -> p c us m", p=128)
            for us in range(NUS):
                nc.sync.dma_start(wx4_sb[:, :, us, :], wx4_r[:, :, us, :])

            # ---- prologue ----
            with (
                tc.tile_pool(name="ptr", bufs=2, space="PSUM") as ptr,
                tc.tile_pool(name="pk4", bufs=2, space="PSUM") as pk4,
                tc.tile_pool(name="pqt", bufs=1, space="PSUM") as pqt,
            ):
                # xq^T first: the qT -> DRAM -> gather chain is the longest
                for dc in range(NDC):
                    tr4 = ptr.tile([128, 512], F32R)
                    for jc in range(NQB):
                        nc.tensor.transpose(
                            tr4[:, ts(jc, 128)],
                            xq_sb[:, jc, ds(dc * 128, 128)],
                            ident_sb[:],
                        )
                    nc.scalar.copy(xqT_sb[:, dc, :], tr4[:])
                qt_ps = pqt.tile([U, LQ], F32)
                for dc in range(NDC):
                    nc.tensor.matmul(
                        qt_ps[:],
                        wt_sb[:, dc, :],
                        xqT_sb[:, dc, :],
                        start=(dc == 0),
                        stop=(dc == NDC - 1),
                    )
                nc.vector.tensor_scalar_add(qtb_sb[:], qt_ps[:], bh_sb[:])
                nc.sync.dma_start(qtb_d.ap(), qtb_sb[:])
                # Qb[us][32uu+ii, g] = qtb[4us+uu, 16ii+g]  (strided groups:
                # group g holds queries {16ii+g}) -> contiguous 64B runs
                qtb_r = qtb_d.ap().rearrange(
                    "(us uu) (ii g) -> uu ii us g", uu=4, g=NGRP
                )
                for uu in range(4):
                    dst = qb_sb[ds(32 * uu, GQ), :, :]
                    nc.sync.dma_start(dst, qtb_r[uu])

                # x^T: 4 chunk-transposes per PSUM tile, one copy per tile
                for n in range(L // 512):
                    for dc in range(NDC):
                        tr4 = ptr.tile([128, 512], F32R)
                        for q4 in range(4):
                            jc = 4 * n + q4
                            nc.tensor.transpose(
                                tr4[:, ts(q4, 128)],
                                x_sb[:, jc, ds(dc * 128, 128)],
                                ident_sb[:],
                            )
                        nc.vector.tensor_copy(
                            xT_sb[:, dc, ds(n * 512, 512)], tr4[:]
                        )

                # K4[us] = k^T slice-replicated, via host-replicated Wx4
                for us in range(NUS):
                    kp = pk4.tile([128, L], F32)
                    for n in range(L // 512):
                        for dc in range(NDC):
                            nc.tensor.matmul(
                                kp[:, ds(n * 512, 512)],
                                wx4_sb[:, dc, us, :],
                                xT_sb[:, dc, ds(n * 512, 512)],
                                start=(dc == 0),
                                stop=(dc == NDC - 1),
                            )
                    nc.scalar.copy(k4_sb[:, us, :], kp[:])

            # ---- main loop ----
            with (
                tc.tile_pool(name="spool", bufs=2) as spool,
                tc.tile_pool(name="hpool", bufs=2) as hpool,
                tc.tile_pool(name="ppool", bufs=2) as ppool,
                tc.tile_pool(name="atpool", bufs=2) as atpool,
                tc.tile_pool(name="vpool", bufs=2) as vpool,
                tc.tile_pool(name="pe", bufs=2, space="PSUM") as pe_e,
                tc.tile_pool(name="pat", bufs=1, space="PSUM") as pe_at,
                tc.tile_pool(name="pv", bufs=2, space="PSUM") as pe_v,
            ):
                out_r = out_d.ap().rearrange(
                    "(ii gg c) d -> gg c ii d", gg=NQB, c=4
                )
                for qb in range(NQB):
                    e_ps = pe_e.tile([128, L], F32)
                    for c in range(4):
                        g = 4 * qb + c
                        for b2 in range(NUS // USB):
                            s = spool.tile([128, USB, L], F32)
                            for k in range(USB):
                                us = USB * b2 + k
                                nc.vector.tensor_scalar_add(
                                    s[:, k, :],
                                    k4_sb[:, us, :],
                                    qb_sb[:, us, ds(g, 1)],
                                )
                            h = hpool.tile([128, USB, L], F16)
                            nc.scalar.activation(h[:], s[:], AF.Tanh)
                            for k in range(USB):
                                us = USB * b2 + k
                                for n in range(L // 512):
                                    nc.tensor.matmul(
                                        e_ps[ds(32 * c, 32), ds(n * 512, 512)],
                                        wa32_sb[:, us, :],
                                        h[:, k, ds(n * 512, 512)],
                                        start=(us == 0),
                                        stop=(us == NUS - 1),
                                        tile_position=(0, 32 * c),
                                    )
                    p = ppool.tile([128, L], F32R)
                    nc.scalar.activation(
                        p[:], e_ps[:], AF.Exp, accum_out=sums_sb[:, ds(qb, 1)]
                    )
                    nc.vector.reciprocal(recip_sb[:, ds(qb, 1)], sums_sb[:, ds(qb, 1)])
                    at_sb = atpool.tile([128, NJC, 128], F32R)
                    at_ps = pe_at.tile([128, L], F32R)
                    for jc in range(NJC):
                        nc.tensor.transpose(
                            at_ps[:, ts(jc, 128)], p[:, ts(jc, 128)], ident_sb[:]
                        )
                    nc.vector.tensor_copy(at_sb[:], at_ps[:])
                    v_ps = pe_v.tile([128, D], F32)
                    for jc in range(NJC):
                        nc.tensor.matmul(
                            v_ps[:],
                            at_sb[:, jc, :],
                            x_sb[:, jc, :],
                            start=(jc == 0),
                            stop=(jc == NJC - 1),
                        )
                    v_sb = vpool.tile([128, D], F32)
                    nc.vector.tensor_scalar_mul(
                        v_sb[:], v_ps[:], recip_sb[:, ds(qb, 1)]
                    )
                    nc.sync.dma_start(out_r[qb], v_sb[:])

    return nc


_NC_CACHE: dict = {}


def get_compiled_nc():
    if "nc" not in _NC_CACHE:
        nc = bacc.Bacc("TRN2", target_bir_lowering=False, debug=False)
        build_kernel(nc)
        nc.compile()
        _NC_CACHE["nc"] = nc
    return _NC_CACHE["nc"]


def make_in_maps(inputs_np, Wt, Wx, bh, Wa):
    wx4 = np.zeros((D, NUS, 128), np.float32)
    wa32 = np.zeros((NUS, 128, GQ), np.float16)
    for us in range(NUS):
        for uu in range(4):
            u = 4 * us + uu
            wx4[:, us, 32 * uu : 32 * (uu + 1)] = Wx[:, u : u + 1]
            wa32[us, 32 * uu : 32 * (uu + 1), :] = Wa[u, 0] * np.eye(GQ, dtype=np.float32)
    bh_c = bh.reshape(U, 1).astype(np.float32)
    ident = np.eye(128, dtype=np.float32)
    in_maps = []
    for c in range(NCORES):
        b, half = divmod(c, HALVES)
        in_maps.append(
            {
                "x": np.ascontiguousarray(inputs_np[b]),
                "xq": np.ascontiguousarray(inputs_np[b, half * LQ : (half + 1) * LQ]),
                "wt": Wt,
                "wx4": wx4,
                "wa32": wa32,
                "bh": bh_c,
                "ident": ident,
            }
        )
    return in_maps


def kernel(**inputs) -> np.ndarray:
    x = np.asarray(inputs["inputs"], dtype=np.float32)
    Wt = np.ascontiguousarray(np.asarray(inputs["Wt"], np.float32))
    Wx = np.ascontiguousarray(np.asarray(inputs["Wx"], np.float32))
    bh = np.asarray(inputs["bh"], np.float32)
    Wa = np.asarray(inputs["Wa"], np.float32)

    from concourse.bass_utils import run_bass_kernel_spmd

    nc = get_compiled_nc()
    in_maps = make_in_maps(x, Wt, Wx, bh, Wa)
    res = run_bass_kernel_spmd(nc, in_maps, list(range(NCORES)))
    kernel._last_results = res  # type: ignore[attr-defined]

    out = np.empty((B, L, D), np.float32)
    for c in range(NCORES):
        b, half = divmod(c, HALVES)
        out[b, half * LQ : (half + 1) * LQ] = res.results[c]["out"]
    return out


# revision 5
# speedup vs baseline: 5360.3080x; 1.0172x over previous
"""Bahdanau additive-attention pooling for TRN2 — v2 layout.

Per core (4 batches x 2 query-halves = 8 cores): 512 queries, 1024 keys.

Partition layout p = 32*uu + ii:  ii in 0..31 = query-within-group,
uu in 0..3 = u-within-slice; u-slices us in 0..7 cover U=32 (u = 4*us+uu).

  K4[us][p, j]  = k[j, 4us+uu]          (built by PE from host-replicated Wx4)
  Qb[us][p, g]  = q[32g+ii, 4us+uu]+bh  (via DRAM round-trip gather of qT)
  S = K4[us] + Qb[us][:,g]              (VectorE tensor_scalar, 2x fp32 mode)
  H = tanh(S)                           (ScalarE, batched 4 slices/instr)
  e[32c:32c+32, :] += wa32[us].T @ H    (PE, M=32 col-tiled at base 32c,
                                         8 accumulating matmuls contract u)
After 4 groups a [128q, 1024k] PSUM logit tile is complete (query block qb).
exp (+row sums via accum_out) -> P; PE-transpose P chunks -> aT; v = aT.T @ x
accumulated over key chunks; scale by 1/rowsum; DMA out.

ba is dropped (softmax shift-invariance); no max-subtraction needed since
|e| <= ||Wa||_1 ~ 4.5.
"""

import numpy as np

import concourse.bass as bass
import concourse.mybir as mybir
import concourse.tile as tile
from concourse import bacc
from concourse.bass import ds, ts

B, L, D, U = 4, 1024, 256, 32
NCORES = 8
HALVES = 2
LQ = L // HALVES                # 512 queries per core
GQ = 32                         # queries per group
NGRP = LQ // GQ                 # 16 groups
NUS = 8                         # u-slices (4 u's each)
USB = 4                         # u-slices per tanh batch
QB = 128                        # query block (softmax granularity)
NQB = LQ // QB                  # 4
NJC = L // 128                  # 8 key chunks
NDC = D // 128                  # 2 contraction chunks

F32 = mybir.dt.float32
F32R = mybir.dt.float32r
F16 = mybir.dt.float16
AF = mybir.ActivationFunctionType


def build_kernel(nc: bass.Bass):
    x_d = nc.dram_tensor("x", [L, D], F32R, kind="ExternalInput")
    xq_d = nc.dram_tensor("xq", [LQ, D], F32R, kind="ExternalInput")
    wt_d = nc.dram_tensor("wt", [D, U], F32R, kind="ExternalInput")
    wx4_d = nc.dram_tensor("wx4", [D, NUS, 128], F32R, kind="ExternalInput")
    wa32_d = nc.dram_tensor("wa32", [NUS, 128, GQ], F16, kind="ExternalInput")
    bh_d = nc.dram_tensor("bh", [U, 1], F32, kind="ExternalInput")
    ident_d = nc.dram_tensor("ident", [128, 128], F32R, kind="ExternalInput")
    out_d = nc.dram_tensor("out", [LQ, D], F32, kind="ExternalOutput")
    qtb_d = nc.dram_tensor("qtb", [U, LQ], F32)  # scratch for the Qb gather

    with tile.TileContext(nc) as tc:
        with tc.tile_pool(name="const", bufs=1) as cpool:
            x_sb = cpool.tile([128, NJC, D], F32R)
            xq_sb = cpool.tile([128, NQB, D], F32R)
            xT_sb = cpool.tile([128, NDC, L], F32R)
            xqT_sb = cpool.tile([128, NDC, LQ], F32R)
            wt_sb = cpool.tile([128, NDC, U], F32R)
            wx4_sb = cpool.tile([128, NDC, NUS, 128], F32R)
            wa32_sb = cpool.tile([128, NUS, GQ], F16)
            bh_sb = cpool.tile([U, 1], F32)
            ident_sb = cpool.tile([128, 128], F32R)
            k4_sb = cpool.tile([128, NUS, L], F32)
            qtb_sb = cpool.tile([U, LQ], F32)
            qb_sb = cpool.tile([128, NUS, NGRP], F32)
            sums_sb = cpool.tile([128, NQB], F32)
            recip_sb = cpool.tile([128, NQB], F32)

            # small/critical DMAs first; 1MB wx4 split per-us and last
            nc.sync.dma_start(ident_sb[:], ident_d.ap())
            nc.sync.dma_start(bh_sb[:], bh_d.ap())
            nc.sync.dma_start(wt_sb[:], wt_d.ap().rearrange("(c p) u -> p c u", p=128))
            nc.sync.dma_start(wa32_sb[:], wa32_d.ap().rearrange("us p m -> p us m"))
            xq_r = xq_d.ap().rearrange("(c p) d -> c p d", p=128)
            for jc in range(NQB):
                nc.sync.dma_start(xq_sb[:, jc, :], xq_r[jc])
            x_r = x_d.ap().rearrange("(c p) d -> c p d", p=128)
            for jc in range(NJC):
                eng = nc.sync if jc % 2 == 0 else nc.gpsimd
                eng.dma_start(x_sb[:, jc, :], x_r[jc])
            wx4_r = wx4_d.ap().rearrange("(c p) us m -> p c us m", p=128)
            for us in range(NUS):
                nc.gpsimd.dma_start(wx4_sb[:, :, us, :], wx4_r[:, :, us, :])

            # ---- prologue ----
            with (
                tc.tile_pool(name="ptr", bufs=3, space="PSUM") as ptr,
                tc.tile_pool(name="pk4", bufs=2, space="PSUM") as pk4,
                tc.tile_pool(name="pqt", bufs=1, space="PSUM") as pqt,
            ):
                # xq^T first: the qT -> DRAM -> gather chain is the longest
                for dc in range(NDC):
                    tr4 = ptr.tile([128, 512], F32R)
                    for jc in range(NQB):
                        nc.tensor.transpose(
                            tr4[:, ts(jc, 128)],
                            xq_sb[:, jc, ds(dc * 128, 128)],
                            ident_sb[:],
                        )
                    nc.scalar.copy(xqT_sb[:, dc, :], tr4[:])
                qt_ps = pqt.tile([U, LQ], F32)
                for dc in range(NDC):
                    nc.tensor.matmul(
                        qt_ps[:],
                        wt_sb[:, dc, :],
                        xqT_sb[:, dc, :],
                        start=(dc == 0),
                        stop=(dc == NDC - 1),
                    )
                nc.vector.tensor_scalar_add(qtb_sb[:], qt_ps[:], bh_sb[:])
                nc.sync.dma_start(qtb_d.ap(), qtb_sb[:])
                # Qb[us][32uu+ii, g] = qtb[4us+uu, 16ii+g]  (strided groups:
                # group g holds queries {16ii+g}) -> contiguous 64B runs
                qtb_r = qtb_d.ap().rearrange(
                    "(us uu) (ii g) -> uu ii us g", uu=4, g=NGRP
                )
                for uu in range(4):
                    dst = qb_sb[ds(32 * uu, GQ), :, :]
                    nc.sync.dma_start(dst, qtb_r[uu])

                # x^T: 4 chunk-transposes per PSUM tile, one copy per tile
                for n in range(L // 512):
                    for dc in range(NDC):
                        tr4 = ptr.tile([128, 512], F32R)
                        for q4 in range(4):
                            jc = 4 * n + q4
                            nc.tensor.transpose(
                                tr4[:, ts(q4, 128)],
                                x_sb[:, jc, ds(dc * 128, 128)],
                                ident_sb[:],
                            )
                        nc.vector.tensor_copy(
                            xT_sb[:, dc, ds(n * 512, 512)], tr4[:]
                        )

                # K4[us] = k^T slice-replicated, via host-replicated Wx4
                for us in range(NUS):
                    kp = pk4.tile([128, L], F32)
                    for n in range(L // 512):
                        for dc in range(NDC):
                            nc.tensor.matmul(
                                kp[:, ds(n * 512, 512)],
                                wx4_sb[:, dc, us, :],
                                xT_sb[:, dc, ds(n * 512, 512)],
                                start=(dc == 0),
                                stop=(dc == NDC - 1),
                            )
                    nc.scalar.copy(k4_sb[:, us, :], kp[:])

            # ---- main loop ----
            with (
                tc.tile_pool(name="spool", bufs=3) as spool,
                tc.tile_pool(name="hpool", bufs=3) as hpool,
                tc.tile_pool(name="ppool", bufs=2) as ppool,
                tc.tile_pool(name="atpool", bufs=2) as atpool,
                tc.tile_pool(name="vpool", bufs=2) as vpool,
                tc.tile_pool(name="pe", bufs=2, space="PSUM") as pe_e,
                tc.tile_pool(name="pat", bufs=1, space="PSUM") as pe_at,
                tc.tile_pool(name="pv", bufs=2, space="PSUM") as pe_v,
            ):
                out_r = out_d.ap().rearrange(
                    "(ii gg c) d -> gg c ii d", gg=NQB, c=4
                )
                for qb in range(NQB):
                    e_ps = pe_e.tile([128, L], F32)
                    for c in range(4):
                        g = 4 * qb + c
                        for b2 in range(NUS // USB):
                            s = spool.tile([128, USB, L], F32)
                            for k in range(USB):
                                us = USB * b2 + k
                                nc.vector.tensor_scalar_add(
                                    s[:, k, :],
                                    k4_sb[:, us, :],
                                    qb_sb[:, us, ds(g, 1)],
                                )
                            h = hpool.tile([128, USB, L], F16)
                            nc.scalar.activation(h[:], s[:], AF.Tanh)
                            for k in range(USB):
                                us = USB * b2 + k
                                for n in range(L // 512):
                                    nc.tensor.matmul(
                                        e_ps[ds(32 * c, 32), ds(n * 512, 512)],
                                        wa32_sb[:, us, :],
                                        h[:, k, ds(n * 512, 512)],
                                        start=(us == 0),
                                        stop=(us == NUS - 1),
                                        tile_position=(0, 32 * c),
                                    )
                    p = ppool.tile([128, L], F32R)
                    nc.scalar.activation(
                        p[:], e_ps[:], AF.Exp, accum_out=sums_sb[:, ds(qb, 1)]
                    )
                    nc.vector.reciprocal(recip_sb[:, ds(qb, 1)], sums_sb[:, ds(qb, 1)])
                    at_sb = atpool.tile([128, NJC, 128], F32R)
                    at_ps = pe_at.tile([128, L], F32R)
                    for jc in range(NJC):
                        nc.tensor.transpose(
                            at_ps[:, ts(jc, 128)], p[:, ts(jc, 128)], ident_sb[:]
                        )
                    nc.vector.tensor_copy(at_sb[:], at_ps[:])
                    v_ps = pe_v.tile([128, D], F32)
                    for jc in range(NJC):
                        nc.tensor.matmul(
                            v_ps[:],
                            at_sb[:, jc, :],
                            x_sb[:, jc, :],
                            start=(jc == 0),
                            stop=(jc == NJC - 1),
                        )
                    v_sb = vpool.tile([128, D], F32)
                    nc.vector.tensor_scalar_mul(
                        v_sb[:], v_ps[:], recip_sb[:, ds(qb, 1)]
                    )
                    nc.sync.dma_start(out_r[qb], v_sb[:])

    return nc


_NC_CACHE: dict = {}


def get_compiled_nc():
    if "nc" not in _NC_CACHE:
        nc = bacc.Bacc("TRN2", target_bir_lowering=False, debug=False)
        build_kernel(nc)
        nc.compile()
        _NC_CACHE["nc"] = nc
    return _NC_CACHE["nc"]


def make_in_maps(inputs_np, Wt, Wx, bh, Wa):
    wx4 = np.zeros((D, NUS, 128), np.float32)
    wa32 = np.zeros((NUS, 128, GQ), np.float16)
    for us in range(NUS):
        for uu in range(4):
            u = 4 * us + uu
            wx4[:, us, 32 * uu : 32 * (uu + 1)] = Wx[:, u : u + 1]
            wa32[us, 32 * uu : 32 * (uu + 1), :] = Wa[u, 0] * np.eye(GQ, dtype=np.float32)
    bh_c = bh.reshape(U, 1).astype(np.float32)
    ident = np.eye(128, dtype=np.float32)
    in_maps = []
    for c in range(NCORES):
        b, half = divmod(c, HALVES)
        in_maps.append(
            {
                "x": np.ascontiguousarray(inputs_np[b]),
                "xq": np.ascontiguousarray(inputs_np[b, half * LQ : (half + 1) * LQ]),
                "wt": Wt,
                "wx4": wx4,
                "wa32": wa32,
                "bh": bh_c,
                "ident": ident,
            }
        )
    return in_maps


def kernel(**inputs) -> np.ndarray:
    x = np.asarray(inputs["inputs"], dtype=np.float32)
    Wt = np.ascontiguousarray(np.asarray(inputs["Wt"], np.float32))
    Wx = np.ascontiguousarray(np.asarray(inputs["Wx"], np.float32))
    bh = np.asarray(inputs["bh"], np.float32)
    Wa = np.asarray(inputs["Wa"], np.float32)

    from concourse.bass_utils import run_bass_kernel_spmd

    nc = get_compiled_nc()
    in_maps = make_in_maps(x, Wt, Wx, bh, Wa)
    res = run_bass_kernel_spmd(nc, in_maps, list(range(NCORES)))
    kernel._last_results = res  # type: ignore[attr-defined]

    out = np.empty((B, L, D), np.float32)
    for c in range(NCORES):
        b, half = divmod(c, HALVES)
        out[b, half * LQ : (half + 1) * LQ] = res.results[c]["out"]
    return out
